# revision 25
# baseline (speedup 1.0000x reference)
"""FPN classifier head (ROI-align + conv head) on 8 Trainium2 NeuronCores.

Self-contained: takes FULL inputs as numpy arrays, shards across 8 cores
(4 ROI-chunks x 2 batches), runs one SPMD Bass program, returns
(logits, probs, bbox) matching reference.py.

Device pipeline per core (250 ROIs of one batch):
  1. indirect-DMA gather of 2x2 bilinear patches from a precomputed
     patch table (one 2KB segment per sample, 128 samples/call, 98 calls)
  2. bilinear combine on DVE (per-partition scalar weights, fused ops)
  3. PE transpose into (channel, roi) layout
  4. conv1 as 98-step accumulated bf16 matmul (K = 49 taps x 256 ch)
  5. BN1 (cross-core AllReduce of sums), relu via ACT
  6. conv2 matmul, BN2 (AllReduce), relu
  7. logits/bbox heads + softmax, one output DMA

Host-side work is limited to input staging (layout transforms of weights
and feature maps, ROI -> gather-index/weight metadata) - all FLOPs over
feature data run on device.
"""
import numpy as np
import ml_dtypes

import concourse.bass as bass
import concourse.mybir as mybir
import concourse.tile as tile
from concourse.alu_op_type import AluOpType
from concourse.bass_utils import run_bass_kernel_spmd
from concourse.tile import add_dep_helper
from concourse.vector_clock import ScopedClock

bf16 = ml_dtypes.bfloat16

# ---------------------------------------------------------------- constants
POOL = 7
IMAGE_SHAPE = (1024, 1024)
NUM_CLASSES = 81
BN_EPS = 1e-5

B, N_ROI, C, F = 2, 1000, 256, 1024
N_CORES = 8
CORES_PER_BATCH = 4
N_PER_CORE = N_ROI // CORES_PER_BATCH          # 250
S = POOL * POOL                                # 49 samples per ROI
NSLOT = 256                                    # padded ROI slots per sample
NCALL_PER_S = NSLOT // 128                     # 2 gather calls per sample
NCALL = S * NCALL_PER_S                        # 98
KHEAD = NUM_CLASSES * 5                        # 405 = 81 logits + 324 bbox
LEVEL_HW = {2: 256, 3: 128, 4: 64, 5: 32}
LEVEL_BASE = {2: 0, 3: 256 * 256, 4: 256 * 256 + 128 * 128,
              5: 256 * 256 + 128 * 128 + 64 * 64}
NPATCH = 256 * 256 + 128 * 128 + 64 * 64 + 32 * 32   # 87040


# ---------------------------------------------------------------- fixups
def _patched_drain_and_barrier(self, tick_clock, wait_clock):
    # Walrus in this image allows only ONE sync wait per TPB instruction.
    # Absorb the exit waits onto sync-engine NOPs preceding the drain.
    nop_inst = self.nc.sync.nop(nofuse=True, hint="tile_exit_waits")
    wait_clock.add_sem_waits(
        nop_inst.ins, ScopedClock({None: tick_clock.global_clock})
    )
    si = nop_inst.ins.sync_info
    waits = list(si.on_wait) if si is not None else []
    if len(waits) > 1:
        si.on_wait = waits[:1]
        for i in range(1, len(waits)):
            extra = self.nc.sync.nop(nofuse=True, hint="tile_exit_waits2")
            esi = extra.ins.sync_info
            if esi is None:
                extra.ins.sync_info = mybir.SyncInfo(
                    on_wait=[waits[i]], on_update=[]
                )
            else:
                esi.on_wait = [waits[i]]
    self.nc.sync.drain()
    self.nc.all_engine_barrier()
    popped = self.nc._tile_sem_poison_stack.pop()
    assert popped is self._sem_poison
    sems = sorted(self.sems.allocated().values(), key=lambda h: h.num)
    for i in range(0, len(sems), 16):
        self.nc.clear_and_free_semaphores(sems[i:i + 16])
    self.nc.all_engine_barrier()


def _split_sem_waits(nc):
    """Hoist >1 sync waits per instruction onto same-engine NOPs."""
    for bb in nc.main_func.blocks:
        insts = bb.instructions
        out = []
        changed = False
        for inst in insts:
            if (type(inst).__name__ == "InstISA"
                    and getattr(inst, "isa_opcode", None) == 176):
                # EVENT_SEMAPHORE_RANGE_CLEAR: kernel-tail sem recycling that
                # this walrus build cannot encode ("ISA wrong length"). Safe
                # to drop: nothing after it in the program reuses those sems,
                # and each launch starts from a freshly loaded NEFF.
                changed = True
                continue
            si = inst.sync_info
            waits = list(si.on_wait) if si is not None else []
            if len(waits) > 1:
                for j, w in enumerate(waits[:-1]):
                    nop = mybir.InstNoOp(
                        name=f"{inst.name}_hw{j}", engine=inst.engine,
                        ins=[], outs=[]
                    )
                    nop.sync_info = mybir.SyncInfo(on_wait=[w], on_update=[])
                    out.append(nop)
                si.on_wait = [waits[-1]]
                changed = True
            out.append(inst)
        if changed:
            bb.instructions = out


tile.TileContext._drain_and_barrier = _patched_drain_and_barrier


# ---------------------------------------------------------------- program
def build_program(replica_groups, bn_count=N_ROI):
    nc = bass.Bass()
    dt = mybir.dt

    ptbl = nc.declare_dram_parameter("ptbl", [NPATCH, 4 * C], dt.bfloat16, isOutput=False)
    idxs_d = nc.declare_dram_parameter("idxs", [128, NCALL], dt.int32, isOutput=False)
    wts_d = nc.declare_dram_parameter("wts", [128, NCALL * 4], dt.float32, isOutput=False)
    w1_d = nc.declare_dram_parameter("w1", [S, 2, 128, F], dt.bfloat16, isOutput=False)
    w2_d = nc.declare_dram_parameter("w2", [8, 8, 128, 128], dt.bfloat16, isOutput=False)
    wh_d = nc.declare_dram_parameter("wh", [8, 128, KHEAD], dt.bfloat16, isOutput=False)
    bnp_d = nc.declare_dram_parameter("bnp", [128, 8, 4], dt.float32, isOutput=False)
    hb_d = nc.declare_dram_parameter("hb", [128, KHEAD], dt.float32, isOutput=False)
    id_d = nc.declare_dram_parameter("idm", [128, 128], dt.bfloat16, isOutput=False)
    out_d = nc.declare_dram_parameter("out", [NSLOT, 2 * NUM_CLASSES + 4 * NUM_CLASSES],
                                      dt.float32, isOutput=True)

    with tile.TileContext(nc) as tc:
        with (
            tc.tile_pool(name="meta", bufs=1) as meta,
            tc.tile_pool(name="gbuf", bufs=1) as gbuf,
            tc.tile_pool(name="plbuf", bufs=4) as plbuf,
            tc.tile_pool(name="trbuf", bufs=3) as trbuf,
            tc.tile_pool(name="w1buf", bufs=4) as w1buf,
            tc.tile_pool(name="wbuf", bufs=4) as wbuf,
            tc.tile_pool(name="whbuf", bufs=1) as whbuf,
            tc.tile_pool(name="res", bufs=1) as res,
            tc.tile_pool(name="small", bufs=2) as small,
            tc.tile_pool(name="ps_tr", bufs=2, space="PSUM") as ps_tr,
            tc.tile_pool(name="ps_x1", bufs=1, space="PSUM") as ps_x1,
            tc.tile_pool(name="dram", bufs=1, space="DRAM") as dram,
        ):
            # ---- resident metadata
            idx_t = meta.tile([128, NCALL], dt.int32)
            idx_ld = nc.sync.dma_start(idx_t[:], idxs_d[:])
            wts_t = meta.tile([128, NCALL * 4], dt.float32)
            nc.sync.dma_start(wts_t[:], wts_d[:])
            bnp_t = meta.tile([128, 8, 4], dt.float32)
            nc.sync.dma_start(bnp_t[:], bnp_d[:])
            hb_t = meta.tile([128, KHEAD], dt.float32)
            nc.sync.dma_start(hb_t[:], hb_d[:])
            ident = meta.tile([128, 128], dt.bfloat16)
            nc.sync.dma_start(ident[:], id_d[:])

            # x1 accumulator: 8 f-chunks packed 2 per PSUM bank
            x1_ps = [ps_x1.tile([128, 512], dt.float32, tag=f"x1b{i}", name=f"x1b{i}") for i in range(4)]

            def x1_slice(fc):
                return x1_ps[fc // 2][:, (fc % 2) * NSLOT:(fc % 2) * NSLOT + NSLOT]

            # ---- main pipeline over 49 samples
            prev_interp = {}     # gbuf slot -> last interp inst (WAR)
            prev_tr_of_pl = {}   # plbuf slot -> last transpose insts (WAR)
            idx_ld_inst = None
            for s in range(S):
                trp = trbuf.tile([128, 2, NSLOT], dt.bfloat16, tag="trp")
                w1_t = w1buf.tile([128, F], dt.bfloat16, tag="w1s")
                nc.sync.dma_start(w1_t[:], w1_d[s, 0, :, :])
                w1_t2 = w1buf.tile([128, F], dt.bfloat16, tag="w1s2")
                nc.sync.dma_start(w1_t2[:], w1_d[s, 1, :, :])
                for j in range(NCALL_PER_S):
                    k = s * NCALL_PER_S + j
                    gslot = k % 4
                    g_t = gbuf.tile([128, 4, C], dt.bfloat16, tag=f"g{gslot}", name=f"g_{s}_{j}")
                    gi = nc.gpsimd.indirect_dma_start(
                        out=g_t[:],
                        out_offset=None,
                        in_=ptbl[:],
                        in_offset=bass.IndirectOffsetOnAxis(
                            ap=idx_t[:, k:k + 1], axis=0),
                    )
                    add_dep_helper(gi.ins, idx_ld.ins, sync=True,
                                   reason="gather RAW idx tile")
                    if gslot in prev_interp:
                        add_dep_helper(gi.ins, prev_interp[gslot], sync=True,
                                       reason="gather WAR on interp")
                    # bilinear combine: pl = sum_j g[:, j, :] * w_j  (per-partition scalars)
                    pl = plbuf.tile([128, C], dt.bfloat16, tag="pl")
                    i0 = nc.vector.tensor_scalar(
                        out=pl[:], in0=g_t[:, 0, :],
                        scalar1=wts_t[:, 4 * k:4 * k + 1], scalar2=None,
                        op0=AluOpType.mult)
                    add_dep_helper(i0.ins, gi.ins, sync=True, reason="interp RAW gather")
                    last = i0
                    for jj in range(1, 4):
                        last = nc.vector.scalar_tensor_tensor(
                            out=pl[:], in0=g_t[:, jj, :],
                            scalar=wts_t[:, 4 * k + jj:4 * k + jj + 1],
                            in1=pl[:], op0=AluOpType.mult, op1=AluOpType.add)
                        add_dep_helper(last.ins, gi.ins, sync=True,
                                       reason="interp RAW gather")
                    prev_interp[gslot] = last.ins
                    # transpose (128 slot, 256 c) -> 2 x (128 c, 128 slot)
                    for h in range(2):
                        pst = ps_tr.tile([128, 128], dt.bfloat16, tag="pst")
                        nc.tensor.transpose(pst[:], pl[:, h * 128:(h + 1) * 128],
                                            ident[:])
                        nc.vector.tensor_copy(
                            trp[:, h, j * 128:(j + 1) * 128], pst[:])
                # conv1 accumulate: out x1[fchunk](128 f, 256 slots)
                for h, wt in ((0, w1_t), (1, w1_t2)):
                    for fc in range(8):
                        nc.tensor.matmul(
                            x1_slice(fc),
                            lhsT=wt[:, fc * 128:(fc + 1) * 128],
                            rhs=trp[:, h, :],
                            start=(s == 0 and h == 0),
                            stop=(s == S - 1 and h == 1),
                        )

            # ---- BN1 stats: per-f sums over valid 250 slots
            stats = res.tile([128, 8, 4], dt.float32)    # sx1, sq1, sx2, sq2
            sq_scr = small.tile([128, N_PER_CORE], dt.float32, tag="sqscr")
            for fc in range(8):
                nc.scalar.activation(
                    sq_scr[:], x1_slice(fc)[:, :N_PER_CORE],
                    mybir.ActivationFunctionType.Copy,
                    accum_out=stats[:, fc, 0:1])
                nc.scalar.activation(
                    sq_scr[:], x1_slice(fc)[:, :N_PER_CORE],
                    mybir.ActivationFunctionType.Square,
                    accum_out=stats[:, fc, 1:2])

            ar_in1 = dram.tile([128, 8, 2], dt.float32)
            ar_out1 = dram.tile([128, 8, 2], dt.float32)
            nc.sync.dma_start(ar_in1[:], stats[:, :, 0:2])
            nc.gpsimd.collective_compute(
                "AllReduce", AluOpType.add, replica_groups=replica_groups,
                ins=[ar_in1.opt()], outs=[ar_out1.opt()])
            st1 = small.tile([128, 8, 2], dt.float32)
            nc.sync.dma_start(st1[:], ar_out1[:])

            # scale/shift in (f-part, chunk) layout:
            # mu = sx/1000 ; var = sq/1000 - mu^2
            # scale = g / sqrt(var+eps) ; shift = b - mu*scale
            sc1 = small.tile([128, 8], dt.float32)
            sh1 = small.tile([128, 8], dt.float32)
            tmp = small.tile([128, 8, 2], dt.float32)

            def bn_coeffs(st, gcol, bcol, sc, sh, tmp):
                inv_n = 1.0 / float(bn_count)
                # tmp0 = mu, tmp1 = E[x^2]
                nc.vector.tensor_scalar(out=tmp[:], in0=st[:],
                                        scalar1=inv_n, scalar2=None,
                                        op0=AluOpType.mult)
                # sc = E[x^2] - mu^2  (var)
                nc.vector.scalar_tensor_tensor(
                    out=sc[:], in0=tmp[:, :, 0], scalar=-1.0,
                    in1=tmp[:, :, 0], op0=AluOpType.mult, op1=AluOpType.mult)
                nc.vector.tensor_tensor(out=sc[:], in0=tmp[:, :, 1], in1=sc[:],
                                        op=AluOpType.add)
                # sc = gamma * exp(-0.5*ln(var+eps))  (rsqrt via ACT LUTs)
                nc.vector.tensor_scalar_add(sc[:], sc[:], BN_EPS)
                nc.scalar.activation(sc[:], sc[:],
                                     mybir.ActivationFunctionType.Ln)
                nc.scalar.activation(sc[:], sc[:],
                                     mybir.ActivationFunctionType.Exp,
                                     scale=-0.5)
                nc.vector.tensor_tensor(out=sc[:], in0=gcol, in1=sc[:],
                                        op=AluOpType.mult)
                # sh = beta - mu * sc
                nc.vector.tensor_tensor(out=sh[:], in0=tmp[:, :, 0], in1=sc[:],
                                        op=AluOpType.mult)
                nc.vector.tensor_tensor(out=sh[:], in0=bcol, in1=sh[:],
                                        op=AluOpType.subtract)

            bn_coeffs(st1, bnp_t[:, :, 0], bnp_t[:, :, 1], sc1, sh1, tmp)

            # ---- BN1 apply + relu -> y1T (f1-part, slots) bf16
            y1 = res.tile([128, 8, NSLOT], dt.bfloat16)
            for fc in range(8):
                nc.scalar.activation(
                    y1[:, fc, :], x1_slice(fc),
                    mybir.ActivationFunctionType.Relu,
                    bias=sh1[:, fc:fc + 1], scale=sc1[:, fc:fc + 1])

            # ---- conv2: x2[f2-part, slots] accumulated over 8 f1 chunks.
            # Reuses the x1 PSUM banks (x1 fully consumed by the bn1 ACT pass;
            # conv2's start=True clears has_written).
            x2_slice = x1_slice

            for i1 in range(8):
                w2_t = wbuf.tile([128, 8, 128], dt.bfloat16, tag="w2")
                nc.sync.dma_start(w2_t[:], w2_d[i1, :, :, :])
                for i2 in range(8):
                    nc.tensor.matmul(
                        x2_slice(i2), lhsT=w2_t[:, i2, :], rhs=y1[:, i1, :],
                        start=(i1 == 0), stop=(i1 == 7))

            # ---- BN2 stats + AllReduce + coeffs
            for fc in range(8):
                nc.scalar.activation(
                    sq_scr[:], x2_slice(fc)[:, :N_PER_CORE],
                    mybir.ActivationFunctionType.Copy,
                    accum_out=stats[:, fc, 2:3])
                nc.scalar.activation(
                    sq_scr[:], x2_slice(fc)[:, :N_PER_CORE],
                    mybir.ActivationFunctionType.Square,
                    accum_out=stats[:, fc, 3:4])
            ar_in2 = dram.tile([128, 8, 2], dt.float32)
            ar_out2 = dram.tile([128, 8, 2], dt.float32)
            nc.sync.dma_start(ar_in2[:], stats[:, :, 2:4])
            nc.gpsimd.collective_compute(
                "AllReduce", AluOpType.add, replica_groups=replica_groups,
                ins=[ar_in2.opt()], outs=[ar_out2.opt()])
            st2 = small.tile([128, 8, 2], dt.float32)
            nc.sync.dma_start(st2[:], ar_out2[:])
            sc2 = small.tile([128, 8], dt.float32)
            sh2 = small.tile([128, 8], dt.float32)
            bn_coeffs(st2, bnp_t[:, :, 2], bnp_t[:, :, 3], sc2, sh2, tmp)

            shared = res.tile([128, 8, NSLOT], dt.bfloat16)
            for fc in range(8):
                nc.scalar.activation(
                    shared[:, fc, :], x2_slice(fc),
                    mybir.ActivationFunctionType.Relu,
                    bias=sh2[:, fc:fc + 1], scale=sc2[:, fc:fc + 1])

            # ---- heads: out (slot-part, 405) = shared^T @ wh, 2 slot chunks
            wh_ts = []
            for fc in range(8):
                wh_t = whbuf.tile([128, KHEAD], dt.bfloat16, tag=f"wh{fc}", name=f"wh{fc}")
                nc.sync.dma_start(wh_t[:], wh_d[fc, :, :])
                wh_ts.append(wh_t)
            for nchunk in range(2):
                hp = ps_tr.tile([128, KHEAD], dt.float32, tag="hps")
                for fc in range(8):
                    nc.tensor.matmul(
                        hp[:],
                        lhsT=shared[:, fc, nchunk * 128:(nchunk + 1) * 128],
                        rhs=wh_ts[fc][:],
                        start=(fc == 0), stop=(fc == 7))
                # add bias
                ho = small.tile([128, 2 * NUM_CLASSES + 4 * NUM_CLASSES],
                                dt.float32, tag="ho")
                lg = small.tile([128, NUM_CLASSES], dt.float32, tag="lg")
                nc.vector.tensor_tensor(out=lg[:], in0=hp[:, :NUM_CLASSES],
                                        in1=hb_t[:, :NUM_CLASSES],
                                        op=AluOpType.add)
                nc.vector.tensor_copy(ho[:, :NUM_CLASSES], lg[:])
                nc.vector.tensor_tensor(
                    out=ho[:, 2 * NUM_CLASSES:],
                    in0=hp[:, NUM_CLASSES:],
                    in1=hb_t[:, NUM_CLASSES:], op=AluOpType.add)
                # softmax on logits
                ex = small.tile([128, NUM_CLASSES], dt.float32, tag="ex")
                esum = small.tile([128, 1], dt.float32, tag="es")
                nc.scalar.activation(ex[:], lg[:],
                                     mybir.ActivationFunctionType.Exp,
                                     accum_out=esum[:])
                rs = small.tile([128, 1], dt.float32, tag="rs")
                nc.scalar.activation(rs[:], esum[:],
                                     mybir.ActivationFunctionType.Ln)
                nc.scalar.activation(rs[:], rs[:],
                                     mybir.ActivationFunctionType.Exp,
                                     scale=-1.0)
                nc.vector.tensor_scalar(
                    out=ho[:, NUM_CLASSES:2 * NUM_CLASSES], in0=ex[:],
                    scalar1=rs[:], scalar2=None, op0=AluOpType.mult)
                nc.sync.dma_start(out_d[nchunk * 128:(nchunk + 1) * 128, :], ho[:])

    _split_sem_waits(nc)
    return nc


# ---------------------------------------------------------------- host prep
def _build_patch_table(feats):
    """feats: list of 4 (C,H,W) fp32 arrays -> (NPATCH, 4*C) bf16 patch table."""
    parts = []
    for f in feats:
        Cc, H, W = f.shape
        hwc = np.ascontiguousarray(f.transpose(1, 2, 0))      # (H, W, C)
        fp = np.zeros((H + 1, W + 1, Cc), np.float32)
        fp[:H, :W] = hwc
        pt = np.concatenate([fp[:H, :W], fp[:H, 1:], fp[1:, :W], fp[1:, 1:]],
                            axis=-1)                          # (H, W, 4C)
        parts.append(pt.reshape(H * W, 4 * Cc))
    return np.concatenate(parts, axis=0).astype(bf16)


def _roi_meta(rois_c):
    """Per-core ROI metadata -> (idx (128,NCALL) i32, wts (128,NCALL*4) f32)."""
    r = rois_c.astype(np.float32)
    h = r[:, 2] - r[:, 0]
    w = r[:, 3] - r[:, 1]
    scale = np.float32(224.0 / np.sqrt(float(IMAGE_SHAPE[0] * IMAGE_SHAPE[1])))
    lvl = np.log2(np.sqrt(np.maximum(h * w, np.float32(1e-12))) / scale)
    level = np.clip(4 + np.round(lvl).astype(np.int32), 2, 5)

    t = np.linspace(0.0, 1.0, POOL, dtype=np.float32)
    n = r.shape[0]
    idx = np.zeros((S, NSLOT), np.int32)
    wts = np.zeros((S, NSLOT, 4), np.float32)
    for i in range(n):
        L = int(level[i])
        H = W = LEVEL_HW[L]
        y1, x1, y2, x2 = r[i]
        ys = (y1 + (y2 - y1) * t) * np.float32(H - 1)
        xs = (x1 + (x2 - x1) * t) * np.float32(W - 1)
        y0 = np.floor(ys)
        x0 = np.floor(xs)
        wy = ys - y0
        wx = xs - x0
        yi = np.clip(y0, 0, H - 1).astype(np.int64)
        xi = np.clip(x0, 0, W - 1).astype(np.int64)
        pid = LEVEL_BASE[L] + yi[:, None] * W + xi[None, :]      # (7py, 7px)
        idx[:, i] = pid.reshape(S)
        w00 = ((1 - wy)[:, None] * (1 - wx)[None, :]).reshape(S)
        w01 = ((1 - wy)[:, None] * wx[None, :]).reshape(S)
        w10 = (wy[:, None] * (1 - wx)[None, :]).reshape(S)
        w11 = (wy[:, None] * wx[None, :]).reshape(S)
        wts[:, i, 0] = w00
        wts[:, i, 1] = w01
        wts[:, i, 2] = w10
        wts[:, i, 3] = w11
    # slot stream -> call-column layout
    idx_calls = idx.reshape(NCALL, 128)                      # call k covers slots
    wts_calls = wts.reshape(NCALL, 128, 4)
    idx_t = np.ascontiguousarray(idx_calls.T)                # (128, NCALL)
    wts_t = np.ascontiguousarray(wts_calls.transpose(1, 0, 2)).reshape(128, NCALL * 4)
    return idx_t, wts_t


def _prep_weights(inp):
    w1 = np.asarray(inp["conv1_w"], np.float32)              # (F, C, 7, 7)
    w1r = w1.transpose(2, 3, 1, 0).reshape(S, C, F)          # (s, c, f)
    w1r = w1r.reshape(S, 2, 128, F).astype(bf16)
    w2 = np.asarray(inp["conv2_w"], np.float32)              # (f2, f1)
    w2r = (w2.T.reshape(8, 128, 8, 128).transpose(0, 2, 1, 3)
           .astype(bf16))                                    # (f1c, f2c, 128, 128)
    lw = np.asarray(inp["logits_w"], np.float32)             # (81, F)
    bw = np.asarray(inp["bbox_w"], np.float32)               # (324, F)
    whead = np.concatenate([lw, bw], axis=0).T               # (F, 405)
    whr = whead.reshape(8, 128, KHEAD).astype(bf16)
    bnp = np.stack([np.asarray(inp["bn1_g"]), np.asarray(inp["bn1_b"]),
                    np.asarray(inp["bn2_g"]), np.asarray(inp["bn2_b"])],
                   axis=-1).astype(np.float32)                # (F, 4)
    bnp = bnp.reshape(8, 128, 4).transpose(1, 0, 2).copy()    # (128, 8, 4)
    hb = np.concatenate([np.asarray(inp["logits_b"]),
                         np.asarray(inp["bbox_b"])]).astype(np.float32)  # (405,)
    hb128 = np.broadcast_to(hb, (128, KHEAD)).copy()
    return w1r, w2r, whr, bnp, hb128


_PROGRAM = None


def _get_program():
    global _PROGRAM
    if _PROGRAM is None:
        _PROGRAM = build_program([[0, 1, 2, 3], [4, 5, 6, 7]])
    return _PROGRAM


def kernel(rois, p2, p3, p4, p5, conv1_w, conv1_b, bn1_g, bn1_b,
           conv2_w, conv2_b, bn2_g, bn2_b, logits_w, logits_b, bbox_w, bbox_b):
    inp = dict(rois=rois, p2=p2, p3=p3, p4=p4, p5=p5, conv1_w=conv1_w,
               conv1_b=conv1_b, bn1_g=bn1_g, bn1_b=bn1_b, conv2_w=conv2_w,
               conv2_b=conv2_b, bn2_g=bn2_g, bn2_b=bn2_b, logits_w=logits_w,
               logits_b=logits_b, bbox_w=bbox_w, bbox_b=bbox_b)
    nc = _get_program()
    w1r, w2r, whr, bnp, hb128 = _prep_weights(inp)
    tbls = [
        _build_patch_table([np.asarray(inp[k], np.float32)[b]
                            for k in ("p2", "p3", "p4", "p5")])
        for b in range(B)
    ]
    rois_f = np.asarray(rois, np.float32)
    in_maps = []
    for core in range(N_CORES):
        b = core // CORES_PER_BATCH
        q = core % CORES_PER_BATCH
        rc = rois_f[b, q * N_PER_CORE:(q + 1) * N_PER_CORE]
        idx_t, wts_t = _roi_meta(rc)
        in_maps.append({
            "ptbl": tbls[b],
            "idxs": idx_t,
            "wts": wts_t,
            "w1": w1r,
            "w2": w2r,
            "wh": whr,
            "bnp": bnp,
            "hb": hb128,
            "idm": np.eye(128, dtype=bf16),
        })
    import os
    trace = bool(os.environ.get("FPN_TRACE"))
    res = run_bass_kernel_spmd(nc, in_maps, list(range(N_CORES)), trace=trace)
    if res.exec_time_ns:
        print(f"HW exec time: {res.exec_time_ns} ns")
    logits = np.zeros((B, N_ROI, NUM_CLASSES), np.float32)
    probs = np.zeros((B, N_ROI, NUM_CLASSES), np.float32)
    bbox = np.zeros((B, N_ROI, NUM_CLASSES, 4), np.float32)
    for core in range(N_CORES):
        b = core // CORES_PER_BATCH
        q = core % CORES_PER_BATCH
        o = res.results[core]["out"][:N_PER_CORE]
        sl = slice(q * N_PER_CORE, (q + 1) * N_PER_CORE)
        logits[b, sl] = o[:, :NUM_CLASSES]
        probs[b, sl] = o[:, NUM_CLASSES:2 * NUM_CLASSES]
        bbox[b, sl] = o[:, 2 * NUM_CLASSES:].reshape(N_PER_CORE, NUM_CLASSES, 4)
    return logits, probs, bbox
